# revision 28
# baseline (speedup 1.0000x reference)
"""Trainium2 Bass kernel for BoundaryOperator SpMM (gnn_message_passing), v4.

out[r, :] = sum over nnz (r, c, v): v * features[c, :].  3M nnz, 500k output
rows, 64 features, 8 cores.

Architecture (row-sharded, SWDGE gather/scatter, no matmul):
  - Core i owns output rows [i*62500, (i+1)*62500) and the ~375k nnz whose
    row lands there.  Output rows split into 2 windows of <=32768 rows so
    scatter indices fit int16.
  - Within a window, nnz are grouped into CELLS by 32768-row feature
    segment (col >> 15), so one `dma_gather` per cell fetches all feature
    rows with int16 indices against a per-cell segment base.
  - Gathered rows are multiplied by the nnz values on DVE (one batched
    tensor_tensor per cell).
  - Each cell's entries are split into occurrence-RANK groups (k-th entry
    of a row in the cell -> group k) so every `dma_scatter_add` has unique
    target rows; scatters are chained on a DMA-completion semaphore so no
    two scatters (which may share rows) are in flight together.
  - Scatter pads point at distinct unused rows with value 0 (adds 0.0).
  - All 8 cores share one instruction grid (per-cell/rank caps = max over
    cores), keeping the program SPMD; per-core index/value streams differ.
  - v4: index/value streams are loaded per CELL in small chunks prefetched
    on the SP queue (the v2 monolithic per-window loads stalled Pool ~20us
    at the start and at the window transition); deeper g/gv pools.

Cost model (measured on CoreSim): every instruction is charged
free_size * CYCLE_T[engine] on its issuing engine; SWDGE gather/scatter
cost 0.4167 ns/idx (64 f32 elements / 128 partitions * 0.833ns) on Pool,
serializing there (~351us busy for totpos=421k); the DVE multiply (~225us)
and all DMA-copy transfers hide underneath.  Notes for future work:
  - uint64-view gathers (32 elems/row = 0.208 ns/idx in the model) produce
    GARBAGE on real HW - the Q7 SWDGE kernel does not support 8B dtypes.
  - The remaining headroom is grid padding (421k vs 375k raw nnz per core):
    per-(cell, rank) caps are max-over-cores rounded to 128.  A doubled
    [F; -F] table gives each nnz two candidate segments (kernel_mirror_wip
    .py): balancing flips can equalize cells, but leftover same-(row,seg)
    collisions still cost ~128-entry rank blocks per cell; eliminating the
    rank machinery entirely requires same-channel (pos mod 16) placement of
    duplicate rows within one scatter - untested on HW.
"""

import os
import sys

import numpy as np

if "/opt/trn_rl_repo" not in sys.path:
    sys.path.append("/opt/trn_rl_repo")

# timing experiments (CoreSim only; numerically wrong when set)
_X_NOZERO = bool(int(os.environ.get("KX_NOZERO", "0")))
_X_NOMULT = bool(int(os.environ.get("KX_NOMULT", "0")))
_X_NOSCAT = bool(int(os.environ.get("KX_NOSCAT", "0")))

P = 128
DF = 64
N_CORES = 8
WIN = 32768          # rows per scatter window (int16 index range)
SEG = 32768          # feature rows per gather segment

_prog_cache: dict = {}


# ---------------------------------------------------------------------------
# Host-side planning
# ---------------------------------------------------------------------------

def _plan(rows, cols, vals, num_out, num_e):
    rows = np.asarray(rows, np.int64)
    cols = np.asarray(cols, np.int64)
    vals = np.asarray(vals, np.float32)
    rows_per_core = num_out // N_CORES
    assert num_out % N_CORES == 0
    n_win = -(-rows_per_core // WIN)
    n_seg = -(-num_e // SEG)

    core = rows // rows_per_core
    lrow = rows - core * rows_per_core
    win = lrow >> 15
    wrow = lrow & (WIN - 1)
    seg = cols >> 15
    segcol = cols & (SEG - 1)

    # Per (core, win, seg) cell: entries with per-row occurrence rank.
    # Cells are split into sub-cells of <= CAPR raw entries to bound the
    # SWDGE descriptor-ring occupancy per instruction.
    CAPR = 10 ** 9   # no sub-cell split (single_packet=False handles big cells)
    order = np.lexsort((rows, seg, win, core))
    c_s, w_s, s_s, r_s = core[order], win[order], seg[order], rows[order]
    sc_s, wr_s, v_s = segcol[order], wrow[order], vals[order]
    idx_all = np.arange(len(order))
    # sub-cell = index within the (core, win, seg) group // CAPR
    cell_change = np.ones(len(order), bool)
    cell_change[1:] = (s_s[1:] != s_s[:-1]) | (w_s[1:] != w_s[:-1]) | \
                      (c_s[1:] != c_s[:-1])
    cell_id = np.cumsum(cell_change) - 1
    cell_starts = np.flatnonzero(cell_change)
    sub_s = (idx_all - cell_starts[cell_id]) // CAPR
    n_sub = int(sub_s.max()) + 1
    # fold sub into the segment key for grid purposes
    k_s = s_s * n_sub + sub_s
    n_key = n_seg * n_sub
    # run starts where (core, win, key, row) changes
    key_change = np.ones(len(order), bool)
    key_change[1:] = (r_s[1:] != r_s[:-1]) | (k_s[1:] != k_s[:-1]) | \
                     (w_s[1:] != w_s[:-1]) | (c_s[1:] != c_s[:-1])
    run_id = np.cumsum(key_change) - 1
    run_starts = np.flatnonzero(key_change)
    rank_s = idx_all - run_starts[run_id]

    # counts[core, win, key, rank]
    max_rank = int(rank_s.max()) + 1
    cnt = np.zeros((N_CORES, n_win, n_key, max_rank), np.int64)
    np.add.at(cnt, (c_s, w_s, k_s, rank_s), 1)

    # Common grid: per (win, key, rank) cap = max over cores, 128-aligned.
    caps = cnt.max(axis=0)                       # [n_win, n_key, max_rank]
    caps = ((caps + P - 1) // P) * P

    # Instruction tables (identical across cores).
    # Per window: list of cells (seg, pos_base, cell_cap, [(rank_cap, pos)]).
    windows = []
    pos = 0
    for w in range(n_win):
        cells = []
        for k in range(n_key):
            rlist = []
            base = pos
            for r in range(max_rank):
                cap = int(caps[w, k, r])
                if cap == 0:
                    break
                rlist.append((pos, cap))
                pos += cap
            if rlist:
                cells.append((k // n_sub, base, pos - base, rlist))
        windows.append(cells)
    totpos = pos
    assert totpos % P == 0

    # Per-core streams.
    gidx = np.zeros((N_CORES, totpos), np.int16)
    sidx = np.zeros((N_CORES, totpos), np.int16)
    valp = np.zeros((N_CORES, totpos), np.float32)

    # Position of each sorted nnz: pos_base(core,w,s,rank) + index-within.
    # Build per-(c,w,s,r) base table from grid caps.
    base_tab = np.zeros((n_win, n_key, max_rank), np.int64)
    pos2 = 0
    for w in range(n_win):
        for k in range(n_key):
            for r in range(max_rank):
                cap = int(caps[w, k, r])
                if cap == 0:
                    break
                base_tab[w, k, r] = pos2
                pos2 += cap
    # index within (c,w,s,r) group: entries are consecutive in sorted order
    # for fixed (c,w,s) with rank increasing within each row-run; regroup via
    # a second sort by (core, win, seg, rank).
    order2 = np.lexsort((rank_s, k_s, w_s, c_s))
    c2, w2, s2, r2 = c_s[order2], w_s[order2], k_s[order2], rank_s[order2]
    sc2, wr2, v2 = sc_s[order2], wr_s[order2], v_s[order2]
    grp_change = np.ones(len(order2), bool)
    grp_change[1:] = (r2[1:] != r2[:-1]) | (s2[1:] != s2[:-1]) | \
                     (w2[1:] != w2[:-1]) | (c2[1:] != c2[:-1])
    g_id = np.cumsum(grp_change) - 1
    g_starts = np.flatnonzero(grp_change)
    within = idx_all - g_starts[g_id]
    p2 = base_tab[w2, s2, r2] + within
    gidx[c2, p2] = sc2
    sidx[c2, p2] = wr2
    valp[c2, p2] = v2

    # Scatter pads: for each (core, win, seg, rank) group, pad slots get
    # distinct rows unused by that group (value stays 0 -> adds 0.0).
    filled = np.zeros((N_CORES, n_win, n_key, max_rank), np.int64)
    np.add.at(filled, (c2, w2, s2, r2), 1)
    for c in range(N_CORES):
        for w in range(n_win):
            for s in range(n_key):
                for r in range(max_rank):
                    cap = int(caps[w, s, r])
                    if cap == 0:
                        break
                    nfill = int(filled[c, w, s, r])
                    npad = cap - nfill
                    if npad == 0:
                        continue
                    b = base_tab[w, s, r]
                    used = sidx[c, b:b + nfill]
                    # candidate pad rows from the top of the window
                    cand = np.arange(WIN - 1, WIN - 1 - nfill - npad - 8, -1,
                                     dtype=np.int64)
                    cand = cand[~np.isin(cand, used)][:npad]
                    sidx[c, b + nfill:b + cap] = cand.astype(np.int16)

    # Wrap streams into device layouts.
    # idx: [128, totpos/16] int16, replicated per 16-partition group.
    g16 = gidx.reshape(N_CORES, totpos // 16, 16).transpose(0, 2, 1)
    s16 = sidx.reshape(N_CORES, totpos // 16, 16).transpose(0, 2, 1)
    gidx_arr = np.tile(g16, (1, 8, 1)).copy()       # [N, 128, totpos/16]
    sidx_arr = np.tile(s16, (1, 8, 1)).copy()
    vals_arr = valp.reshape(N_CORES, totpos // P, P).transpose(0, 2, 1).copy()

    plan = {
        "windows": windows,
        "totpos": totpos,
        "n_win": n_win,
        "rows_per_core": rows_per_core,
        "cellcap_max": max((cc for cells in windows for _, _, cc, _ in cells),
                           default=P),
        "win_pos": [  # per window: (pos_base, npos)
        ],
    }
    for w, cells in enumerate(windows):
        if cells:
            b0 = cells[0][1]
            b1 = cells[-1][1] + cells[-1][2]
            plan["win_pos"].append((b0, b1 - b0))
        else:
            plan["win_pos"].append((0, 0))
    return plan, gidx_arr, sidx_arr, vals_arr


# ---------------------------------------------------------------------------
# Bass program
# ---------------------------------------------------------------------------

def _build_program(num_e, plan):
    import concourse.bacc as bacc
    import concourse.bass as bass
    import concourse.mybir as mybir
    from concourse.tile import TileContext

    f32, i16 = mybir.dt.float32, mybir.dt.int16
    totpos = plan["totpos"]
    windows = plan["windows"]
    cellmax = plan["cellcap_max"]
    out_rows = 2 * WIN      # window 1 scatter range may reach 65535
    zero_rows = ((plan["rows_per_core"] + P - 1) // P) * P

    nc = bacc.Bacc(None, target_bir_lowering=False,
                   dynamic_dma_scratch_size=49152)
    feat = nc.dram_tensor("features", [num_e, DF], f32, kind="ExternalInput")
    gidxd = nc.dram_tensor("gidx", [P, totpos // 16], i16, kind="ExternalInput")
    sidxd = nc.dram_tensor("sidx", [P, totpos // 16], i16, kind="ExternalInput")
    valsd = nc.dram_tensor("vals", [P, totpos // P], f32, kind="ExternalInput")
    outw = [nc.dram_tensor(f"out_w{w}", [WIN, DF], f32, kind="ExternalOutput")
            for w in range(len(windows))]

    GB = 8     # gather-tile pool depth
    GVB = 6    # multiplied-tile pool depth (slots read by scatter DMAs)
    SB = 12    # per-cell stream-chunk pool depth
    with TileContext(nc) as tc:
        with (
            tc.tile_pool(name="z", bufs=1) as zpool,
            tc.tile_pool(name="idx", bufs=SB) as ipool,
            tc.tile_pool(name="g", bufs=GB) as gpool,
            tc.tile_pool(name="gv", bufs=GVB) as gvpool,
        ):
            # Zero the real output rows (Act engine, big contiguous DMAs).
            # Per-window tensors: window 1 scatters only wait on window-1
            # zeroing, which overlaps with window-0 processing.
            z_t = zpool.tile([P, 2048], f32)
            nc.vector.memset(z_t[:], 0.0)
            zrem = zero_rows
            for w in range(len(windows)):
                zr = min(WIN, ((zrem + P - 1) // P) * P)
                if zr <= 0:
                    break
                zrem -= zr
                zc = (zr // P) * DF
                ov = outw[w].ap()[0:zr, :].rearrange("(p c) f -> p (c f)", p=P)
                for o in range(0, zc, 2048):
                    ww = min(2048, zc - o)
                    if not _X_NOZERO:
                        nc.scalar.dma_start(out=ov[:, o:o + ww], in_=z_t[:, :ww])

            scat_sem = nc.alloc_semaphore("scat_dma")
            n_scat = 0

            # Software pipeline: gather(cell i+1) is emitted before the
            # scatters of cell i so the Pool engine overlaps gathers with
            # scatter sem-waits.  Index/value streams are loaded per CELL
            # (small chunks, prefetched PF cells ahead on the SP queue) so
            # the Pool engine never stalls on a monolithic window load.
            all_cells = [(w, cell) for w, cells in enumerate(windows)
                         for cell in cells]
            cell_tiles = {}
            gv_tiles = {}
            cum_scat = {}   # cell index -> total scatters emitted through it
            PF = SB - 2

            def emit_loads(ci):
                w, (s, base, cap, rlist) = all_cells[ci]
                # stream-chunk slot reused from cell ci-SB: six/val are read
                # by that cell's scatter DMAs -> gate the overwrite on their
                # completion (SP engine wait; SP is otherwise idle).
                if ci >= SB and cum_scat[ci - SB] > 0:
                    nc.sync.wait_ge(scat_sem, 16 * cum_scat[ci - SB])
                gix_t = ipool.tile([P, cap // 16], i16, tag="gix")
                nc.sync.dma_start(out=gix_t[:],
                                  in_=gidxd[:, base // 16:(base + cap) // 16])
                six_t = ipool.tile([P, cap // 16], i16, tag="six")
                nc.sync.dma_start(out=six_t[:],
                                  in_=sidxd[:, base // 16:(base + cap) // 16])
                val_t = ipool.tile([P, cap // P], f32, tag="val")
                nc.sync.dma_start(out=val_t[:],
                                  in_=valsd[:, base // P:(base + cap) // P])
                cell_tiles[ci] = (gix_t, six_t, val_t)

            def emit_gather(ci):
                w, (s, base, cap, rlist) = all_cells[ci]
                gix_t, six_t, val_t = cell_tiles[ci]
                # gv slot reused by cell ci from cell ci-GVB, whose scatter
                # DMAs read it asynchronously: gate the DVE multiply on their
                # completion.
                if ci >= GVB:
                    nc.vector.wait_ge(scat_sem, 16 * cum_scat[ci - GVB])
                ncol = cap // P
                g_t = gpool.tile([P, (cellmax // P) * DF], f32, tag="g")
                seg_lo = s * SEG
                seg_hi = min(seg_lo + SEG, num_e)
                nc.gpsimd.dma_gather(
                    out_ap=g_t[:, :ncol * DF].rearrange("p (c f) -> p c f",
                                                        f=DF),
                    in_ap=feat[seg_lo:seg_hi, :],
                    idxs_ap=gix_t[:],
                    num_idxs=cap,
                    num_idxs_reg=cap,
                    elem_size=DF,
                    single_packet=False,
                )
                g_f32 = g_t[:]
                if _X_NOMULT:
                    gv_tiles[ci] = g_t
                    return
                gv_t = gvpool.tile([P, (cellmax // P) * DF], f32, tag="gv")
                nc.vector.tensor_tensor(
                    out=gv_t[:, :ncol * DF].rearrange("p (c f) -> p c f", f=DF),
                    in0=g_f32[:, :ncol * DF].rearrange("p (c f) -> p c f", f=DF),
                    in1=val_t[:].to_broadcast([P, ncol, DF]),
                    op=mybir.AluOpType.mult,
                )
                gv_tiles[ci] = gv_t

            def emit_scatters(ci):
                nonlocal n_scat
                w, (s, base, cap, rlist) = all_cells[ci]
                gix_t, six_t, val_t = cell_tiles[ci]
                gv_t = gv_tiles.pop(ci)
                if _X_NOSCAT:
                    cum_scat[ci] = n_scat
                    return
                if _X_NOMULT:
                    gv_t = gv_t[:].bitcast(f32)
                for (rpos, rcap) in rlist:
                    o = rpos - base
                    prev = n_scat
                    nc.gpsimd.dma_scatter_add(
                        out_ap=outw[w][:, :],
                        in_ap=gv_t[:, (o // P) * DF:((o + rcap) // P) * DF]
                            .rearrange("p (c f) -> p c f", f=DF),
                        idxs_ap=six_t[:, o // 16:(o + rcap) // 16],
                        num_idxs=rcap,
                        num_idxs_reg=rcap,
                        elem_size=DF,
                        single_packet=False,
                    ).then_inc(scat_sem, 16)._maybe_wait_ge(
                        (scat_sem, 16 * prev) if prev > 0 else None)
                    n_scat += 1
                cum_scat[ci] = n_scat

            n_cells = len(all_cells)
            next_load = 0
            for ci in range(n_cells):
                while next_load <= min(ci + PF, n_cells - 1):
                    emit_loads(next_load)
                    next_load += 1
                emit_gather(ci)
                if ci > 0:
                    emit_scatters(ci - 1)
            if all_cells:
                emit_scatters(n_cells - 1)

    nc.compile()
    return nc


def _grid_key(plan):
    return (plan["totpos"],
            tuple((w, s, base, cap, tuple(rl))
                  for w, cells in enumerate(plan["windows"])
                  for (s, base, cap, rl) in cells))


def _get_program(num_e, plan):
    key = (num_e, _grid_key(plan))
    if key not in _prog_cache:
        _prog_cache[key] = _build_program(num_e, plan)
    return _prog_cache[key]


# ---------------------------------------------------------------------------
# Entry point
# ---------------------------------------------------------------------------

def kernel(simplex_features, boundary_values, boundary_rows, boundary_cols,
           num_out, _trace=False):
    from concourse.bass_utils import run_bass_kernel_spmd

    num_out = int(num_out)
    feats = np.ascontiguousarray(np.asarray(simplex_features, np.float32))
    num_e = feats.shape[0]

    plan, gidx_arr, sidx_arr, vals_arr = _plan(
        np.asarray(boundary_rows), np.asarray(boundary_cols),
        np.asarray(boundary_values), num_out, num_e)

    nc = _get_program(num_e, plan)

    in_maps = [
        {
            "features": feats,
            "gidx": np.ascontiguousarray(gidx_arr[i]),
            "sidx": np.ascontiguousarray(sidx_arr[i]),
            "vals": np.ascontiguousarray(vals_arr[i]),
        }
        for i in range(N_CORES)
    ]
    res = run_bass_kernel_spmd(nc, in_maps, list(range(N_CORES)), trace=_trace)

    rpc = plan["rows_per_core"]
    out = np.empty((num_out, DF), np.float32)
    for i in range(N_CORES):
        parts = []
        rem = rpc
        for w in range(plan["n_win"]):
            take = min(WIN, rem)
            parts.append(np.asarray(res.results[i][f"out_w{w}"])[:take])
            rem -= take
        out[i * rpc:(i + 1) * rpc] = np.concatenate(parts, axis=0)
    if _trace:
        return out, res
    return out


def estimate_core_time_ns(simplex_features, boundary_values, boundary_rows,
                          boundary_cols, num_out):
    """Cost-model span (ns) of one core's program via no-exec CoreSim."""
    from concourse.bass_interp import CoreSim

    num_out = int(num_out)
    num_e = np.asarray(simplex_features).shape[0]
    plan, _, _, _ = _plan(
        np.asarray(boundary_rows), np.asarray(boundary_cols),
        np.asarray(boundary_values), num_out, num_e)
    nc = _get_program(num_e, plan)
    sim = CoreSim(nc, no_exec=True, publish_trace=False)
    sim.simulate()
    return int(sim.time)



# revision 33
# speedup vs baseline: 1.0187x; 1.0187x over previous
"""Trainium2 Bass kernel for BoundaryOperator SpMM (gnn_message_passing), v4.

out[r, :] = sum over nnz (r, c, v): v * features[c, :].  3M nnz, 500k output
rows, 64 features, 8 cores.

Architecture (row-sharded, SWDGE gather/scatter, no matmul):
  - Core i owns output rows [i*62500, (i+1)*62500) and the ~375k nnz whose
    row lands there.  Output rows split into 2 windows of <=32768 rows so
    scatter indices fit int16.
  - Within a window, nnz are grouped into CELLS by 32768-row feature
    segment (col >> 15), so one `dma_gather` per cell fetches all feature
    rows with int16 indices against a per-cell segment base.
  - Gathered rows are multiplied by the nnz values on DVE (one batched
    tensor_tensor per cell).
  - Each cell's entries are split into occurrence-RANK groups (k-th entry
    of a row in the cell -> group k) so every `dma_scatter_add` has unique
    target rows; scatters are chained on a DMA-completion semaphore so no
    two scatters (which may share rows) are in flight together.
  - Scatter pads point at distinct unused rows with value 0 (adds 0.0).
  - All 8 cores share one instruction grid (per-cell/rank caps = max over
    cores), keeping the program SPMD; per-core index/value streams differ.
  - v4: index/value streams are loaded per CELL in small chunks prefetched
    on the SP queue (the v2 monolithic per-window loads stalled Pool ~20us
    at the start and at the window transition); deeper g/gv pools.

Cost model (measured on CoreSim): every instruction is charged
free_size * CYCLE_T[engine] on its issuing engine; SWDGE gather/scatter
cost 0.4167 ns/idx (64 f32 elements / 128 partitions * 0.833ns) on Pool,
serializing there (~351us busy for totpos=421k); the DVE multiply (~225us)
and all DMA-copy transfers hide underneath.  Notes for future work:
  - uint64-view gathers (32 elems/row = 0.208 ns/idx in the model) produce
    GARBAGE on real HW - the Q7 SWDGE kernel does not support 8B dtypes.
  - The remaining headroom is grid padding (421k vs 375k raw nnz per core):
    per-(cell, rank) caps are max-over-cores rounded to 128.  A doubled
    [F; -F] table gives each nnz two candidate segments (kernel_mirror_wip
    .py): balancing flips can equalize cells, but leftover same-(row,seg)
    collisions still cost ~128-entry rank blocks per cell; eliminating the
    rank machinery entirely requires same-channel (pos mod 16) placement of
    duplicate rows within one scatter - untested on HW.
"""

import os
import sys

import numpy as np

if "/opt/trn_rl_repo" not in sys.path:
    sys.path.append("/opt/trn_rl_repo")

# timing experiments (CoreSim only; numerically wrong when set)
_X_NOZERO = bool(int(os.environ.get("KX_NOZERO", "0")))
_X_NOMULT = bool(int(os.environ.get("KX_NOMULT", "0")))
_X_NOSCAT = bool(int(os.environ.get("KX_NOSCAT", "0")))

P = 128
DF = 64
N_CORES = 8
WIN = 32768          # rows per scatter window (int16 index range)
SEG = 32768          # feature rows per gather segment

_prog_cache: dict = {}


# ---------------------------------------------------------------------------
# Host-side planning
# ---------------------------------------------------------------------------

def _balance_rows(rows, seg, num_out, rows_per_core, n_win, n_seg):
    """Assign output rows to (core, win) bins equalizing per-(win, seg)
    cell counts across cores.

    The SPMD grid caps each (win, seg, rank) block at the max count over
    cores (rounded to 128); random row->core assignment costs ~+3 sigma of
    padding per cell.  Swapping rows between over- and under-loaded cores
    (same window, preserving bin sizes) flattens the per-seg counts, so the
    shared caps approach the mean.  Returns (core_of_row, win_of_row,
    wrow_of_row, row_ids_per_core) with row_ids in window-major order.
    """
    win_sizes = [min(WIN, rows_per_core - w * WIN) for w in range(n_win)]
    # natural init
    r_all = np.arange(num_out, dtype=np.int64)
    core_of = r_all // rows_per_core
    lrow = r_all - core_of * rows_per_core
    win_of = lrow >> 15
    bin_of = core_of * n_win + win_of
    nbins = N_CORES * n_win

    deg = np.bincount(rows, minlength=num_out)
    low_deg = deg <= 3          # swap candidates: little collateral noise
    rng = np.random.default_rng(99)
    for rnd in range(24):
        bid = bin_of[rows]
        cnt = np.bincount(bid * n_seg + seg,
                          minlength=nbins * n_seg).reshape(nbins, n_seg)
        moved = 0
        for w in range(n_win):
            bins = np.arange(N_CORES) * n_win + w
            M = cnt[bins]
            spread = M.max(axis=0) - M.min(axis=0)
            for s in np.argsort(-spread)[:24]:
                gap = int(spread[s])
                if gap < 8:
                    break
                ob = int(bins[M[:, s].argmax()])
                ub = int(bins[M[:, s].argmin()])
                k = min(gap // 2, 96)
                # low-degree rows in the over-bin touching seg s
                em = (seg == s) & (bid == ob)
                cand_o = np.unique(rows[em])
                # bid is stale within a round: keep only rows still in ob
                cand_o = cand_o[low_deg[cand_o] & (bin_of[cand_o] == ob)]
                if len(cand_o) == 0:
                    continue
                k = min(k, len(cand_o))
                sel_o = rng.choice(cand_o, size=k, replace=False)
                # low-degree rows in the under-bin not touching seg s
                rows_u = np.flatnonzero((bin_of == ub) & low_deg)
                with_s = np.unique(rows[(seg == s) & (bid == ub)])
                cand_u = rows_u[~np.isin(rows_u, with_s)]
                if len(cand_u) < k:
                    k = len(cand_u)
                    sel_o = sel_o[:k]
                if k == 0:
                    continue
                sel_u = rng.choice(cand_u, size=k, replace=False)
                bin_of[sel_o] = ub
                bin_of[sel_u] = ob
                moved += k
                # counts refresh at the top of the next round; stale M
                # within a round only mildly misdirects later segs
        if moved == 0:
            break

    core_of = bin_of // n_win
    win_of = bin_of - core_of * n_win
    wrow_of = np.zeros(num_out, np.int64)
    row_ids_per_core = []
    for c in range(N_CORES):
        ids_c = []
        for w in range(n_win):
            ids = np.flatnonzero(bin_of == c * n_win + w)
            assert len(ids) == win_sizes[w], (c, w, len(ids), win_sizes[w])
            wrow_of[ids] = np.arange(len(ids))
            ids_c.append(ids)
        row_ids_per_core.append(np.concatenate(ids_c))
    return core_of, win_of, wrow_of, row_ids_per_core


def _plan(rows, cols, vals, num_out, num_e):
    rows = np.asarray(rows, np.int64)
    cols = np.asarray(cols, np.int64)
    vals = np.asarray(vals, np.float32)
    rows_per_core = num_out // N_CORES
    assert num_out % N_CORES == 0
    n_win = -(-rows_per_core // WIN)
    n_seg = -(-num_e // SEG)

    seg = cols >> 15
    segcol = cols & (SEG - 1)
    core_of, win_of, wrow_of, row_ids_per_core = _balance_rows(
        rows, seg, num_out, rows_per_core, n_win, n_seg)
    core = core_of[rows]
    win = win_of[rows]
    wrow = wrow_of[rows]

    # Per (core, win, seg) cell: entries with per-row occurrence rank.
    # Cells are split into sub-cells of <= CAPR raw entries to bound the
    # SWDGE descriptor-ring occupancy per instruction.
    CAPR = 10 ** 9   # no sub-cell split (single_packet=False handles big cells)
    order = np.lexsort((rows, seg, win, core))
    c_s, w_s, s_s, r_s = core[order], win[order], seg[order], rows[order]
    sc_s, wr_s, v_s = segcol[order], wrow[order], vals[order]
    idx_all = np.arange(len(order))
    # sub-cell = index within the (core, win, seg) group // CAPR
    cell_change = np.ones(len(order), bool)
    cell_change[1:] = (s_s[1:] != s_s[:-1]) | (w_s[1:] != w_s[:-1]) | \
                      (c_s[1:] != c_s[:-1])
    cell_id = np.cumsum(cell_change) - 1
    cell_starts = np.flatnonzero(cell_change)
    sub_s = (idx_all - cell_starts[cell_id]) // CAPR
    n_sub = int(sub_s.max()) + 1
    # fold sub into the segment key for grid purposes
    k_s = s_s * n_sub + sub_s
    n_key = n_seg * n_sub
    # run starts where (core, win, key, row) changes
    key_change = np.ones(len(order), bool)
    key_change[1:] = (r_s[1:] != r_s[:-1]) | (k_s[1:] != k_s[:-1]) | \
                     (w_s[1:] != w_s[:-1]) | (c_s[1:] != c_s[:-1])
    run_id = np.cumsum(key_change) - 1
    run_starts = np.flatnonzero(key_change)
    rank_s = idx_all - run_starts[run_id]

    # counts[core, win, key, rank]
    max_rank = int(rank_s.max()) + 1
    cnt = np.zeros((N_CORES, n_win, n_key, max_rank), np.int64)
    np.add.at(cnt, (c_s, w_s, k_s, rank_s), 1)

    # Common grid: per (win, key, rank) cap = max over cores, 128-aligned.
    caps = cnt.max(axis=0)                       # [n_win, n_key, max_rank]
    caps = ((caps + P - 1) // P) * P

    # Instruction tables (identical across cores).
    # Per window: list of cells (seg, pos_base, cell_cap, [(rank_cap, pos)]).
    windows = []
    pos = 0
    for w in range(n_win):
        cells = []
        for k in range(n_key):
            rlist = []
            base = pos
            for r in range(max_rank):
                cap = int(caps[w, k, r])
                if cap == 0:
                    break
                rlist.append((pos, cap))
                pos += cap
            if rlist:
                cells.append((k // n_sub, base, pos - base, rlist))
        windows.append(cells)
    totpos = pos
    assert totpos % P == 0

    # Per-core streams.
    gidx = np.zeros((N_CORES, totpos), np.int16)
    sidx = np.zeros((N_CORES, totpos), np.int16)
    valp = np.zeros((N_CORES, totpos), np.float32)

    # Position of each sorted nnz: pos_base(core,w,s,rank) + index-within.
    # Build per-(c,w,s,r) base table from grid caps.
    base_tab = np.zeros((n_win, n_key, max_rank), np.int64)
    pos2 = 0
    for w in range(n_win):
        for k in range(n_key):
            for r in range(max_rank):
                cap = int(caps[w, k, r])
                if cap == 0:
                    break
                base_tab[w, k, r] = pos2
                pos2 += cap
    # index within (c,w,s,r) group: entries are consecutive in sorted order
    # for fixed (c,w,s) with rank increasing within each row-run; regroup via
    # a second sort by (core, win, seg, rank).
    order2 = np.lexsort((rank_s, k_s, w_s, c_s))
    c2, w2, s2, r2 = c_s[order2], w_s[order2], k_s[order2], rank_s[order2]
    sc2, wr2, v2 = sc_s[order2], wr_s[order2], v_s[order2]
    grp_change = np.ones(len(order2), bool)
    grp_change[1:] = (r2[1:] != r2[:-1]) | (s2[1:] != s2[:-1]) | \
                     (w2[1:] != w2[:-1]) | (c2[1:] != c2[:-1])
    g_id = np.cumsum(grp_change) - 1
    g_starts = np.flatnonzero(grp_change)
    within = idx_all - g_starts[g_id]
    p2 = base_tab[w2, s2, r2] + within
    gidx[c2, p2] = sc2
    sidx[c2, p2] = wr2
    valp[c2, p2] = v2

    # Scatter pads: for each (core, win, seg, rank) group, pad slots get
    # distinct rows unused by that group (value stays 0 -> adds 0.0).
    filled = np.zeros((N_CORES, n_win, n_key, max_rank), np.int64)
    np.add.at(filled, (c2, w2, s2, r2), 1)
    for c in range(N_CORES):
        for w in range(n_win):
            for s in range(n_key):
                for r in range(max_rank):
                    cap = int(caps[w, s, r])
                    if cap == 0:
                        break
                    nfill = int(filled[c, w, s, r])
                    npad = cap - nfill
                    if npad == 0:
                        continue
                    b = base_tab[w, s, r]
                    used = sidx[c, b:b + nfill]
                    # candidate pad rows from the top of the window
                    cand = np.arange(WIN - 1, WIN - 1 - nfill - npad - 8, -1,
                                     dtype=np.int64)
                    cand = cand[~np.isin(cand, used)][:npad]
                    sidx[c, b + nfill:b + cap] = cand.astype(np.int16)

    # Wrap streams into device layouts.
    # idx: [128, totpos/16] int16, replicated per 16-partition group.
    g16 = gidx.reshape(N_CORES, totpos // 16, 16).transpose(0, 2, 1)
    s16 = sidx.reshape(N_CORES, totpos // 16, 16).transpose(0, 2, 1)
    gidx_arr = np.tile(g16, (1, 8, 1)).copy()       # [N, 128, totpos/16]
    sidx_arr = np.tile(s16, (1, 8, 1)).copy()
    vals_arr = valp.reshape(N_CORES, totpos // P, P).transpose(0, 2, 1).copy()

    plan = {
        "windows": windows,
        "totpos": totpos,
        "n_win": n_win,
        "rows_per_core": rows_per_core,
        "row_ids_per_core": row_ids_per_core,
        "cellcap_max": max((cc for cells in windows for _, _, cc, _ in cells),
                           default=P),
        "win_pos": [  # per window: (pos_base, npos)
        ],
    }
    for w, cells in enumerate(windows):
        if cells:
            b0 = cells[0][1]
            b1 = cells[-1][1] + cells[-1][2]
            plan["win_pos"].append((b0, b1 - b0))
        else:
            plan["win_pos"].append((0, 0))
    return plan, gidx_arr, sidx_arr, vals_arr


# ---------------------------------------------------------------------------
# Bass program
# ---------------------------------------------------------------------------

def _build_program(num_e, plan):
    import concourse.bacc as bacc
    import concourse.bass as bass
    import concourse.mybir as mybir
    from concourse.tile import TileContext

    f32, i16 = mybir.dt.float32, mybir.dt.int16
    totpos = plan["totpos"]
    windows = plan["windows"]
    cellmax = plan["cellcap_max"]
    out_rows = 2 * WIN      # window 1 scatter range may reach 65535
    zero_rows = ((plan["rows_per_core"] + P - 1) // P) * P

    nc = bacc.Bacc(None, target_bir_lowering=False,
                   dynamic_dma_scratch_size=49152)
    feat = nc.dram_tensor("features", [num_e, DF], f32, kind="ExternalInput")
    gidxd = nc.dram_tensor("gidx", [P, totpos // 16], i16, kind="ExternalInput")
    sidxd = nc.dram_tensor("sidx", [P, totpos // 16], i16, kind="ExternalInput")
    valsd = nc.dram_tensor("vals", [P, totpos // P], f32, kind="ExternalInput")
    outw = [nc.dram_tensor(f"out_w{w}", [WIN, DF], f32, kind="ExternalOutput")
            for w in range(len(windows))]

    GB = 8     # gather-tile pool depth
    GVB = 6    # multiplied-tile pool depth (slots read by scatter DMAs)
    SB = 12    # per-cell stream-chunk pool depth
    with TileContext(nc) as tc:
        with (
            tc.tile_pool(name="z", bufs=1) as zpool,
            tc.tile_pool(name="idx", bufs=SB) as ipool,
            tc.tile_pool(name="g", bufs=GB) as gpool,
            tc.tile_pool(name="gv", bufs=GVB) as gvpool,
        ):
            # Zero the real output rows (Act engine, big contiguous DMAs).
            # Per-window tensors: window 1 scatters only wait on window-1
            # zeroing, which overlaps with window-0 processing.
            z_t = zpool.tile([P, 2048], f32)
            nc.vector.memset(z_t[:], 0.0)
            zrem = zero_rows
            for w in range(len(windows)):
                zr = min(WIN, ((zrem + P - 1) // P) * P)
                if zr <= 0:
                    break
                zrem -= zr
                zc = (zr // P) * DF
                ov = outw[w].ap()[0:zr, :].rearrange("(p c) f -> p (c f)", p=P)
                for o in range(0, zc, 2048):
                    ww = min(2048, zc - o)
                    if not _X_NOZERO:
                        nc.scalar.dma_start(out=ov[:, o:o + ww], in_=z_t[:, :ww])

            scat_sem = nc.alloc_semaphore("scat_dma")
            n_scat = 0

            # Software pipeline: gather(cell i+1) is emitted before the
            # scatters of cell i so the Pool engine overlaps gathers with
            # scatter sem-waits.  Index/value streams are loaded per CELL
            # (small chunks, prefetched PF cells ahead on the SP queue) so
            # the Pool engine never stalls on a monolithic window load.
            all_cells = [(w, cell) for w, cells in enumerate(windows)
                         for cell in cells]
            cell_tiles = {}
            gv_tiles = {}
            cum_scat = {}   # cell index -> total scatters emitted through it
            PF = SB - 2

            def emit_loads(ci):
                w, (s, base, cap, rlist) = all_cells[ci]
                # stream-chunk slot reused from cell ci-SB: six/val are read
                # by that cell's scatter DMAs -> gate the overwrite on their
                # completion (SP engine wait; SP is otherwise idle).
                if ci >= SB and cum_scat[ci - SB] > 0:
                    nc.sync.wait_ge(scat_sem, 16 * cum_scat[ci - SB])
                gix_t = ipool.tile([P, cap // 16], i16, tag="gix")
                nc.sync.dma_start(out=gix_t[:],
                                  in_=gidxd[:, base // 16:(base + cap) // 16])
                six_t = ipool.tile([P, cap // 16], i16, tag="six")
                nc.sync.dma_start(out=six_t[:],
                                  in_=sidxd[:, base // 16:(base + cap) // 16])
                val_t = ipool.tile([P, cap // P], f32, tag="val")
                nc.sync.dma_start(out=val_t[:],
                                  in_=valsd[:, base // P:(base + cap) // P])
                cell_tiles[ci] = (gix_t, six_t, val_t)

            def emit_gather(ci):
                w, (s, base, cap, rlist) = all_cells[ci]
                gix_t, six_t, val_t = cell_tiles[ci]
                # gv slot reused by cell ci from cell ci-GVB, whose scatter
                # DMAs read it asynchronously: gate the DVE multiply on their
                # completion.
                if ci >= GVB:
                    nc.vector.wait_ge(scat_sem, 16 * cum_scat[ci - GVB])
                ncol = cap // P
                g_t = gpool.tile([P, (cellmax // P) * DF], f32, tag="g")
                seg_lo = s * SEG
                seg_hi = min(seg_lo + SEG, num_e)
                nc.gpsimd.dma_gather(
                    out_ap=g_t[:, :ncol * DF].rearrange("p (c f) -> p c f",
                                                        f=DF),
                    in_ap=feat[seg_lo:seg_hi, :],
                    idxs_ap=gix_t[:],
                    num_idxs=cap,
                    num_idxs_reg=cap,
                    elem_size=DF,
                    single_packet=False,
                )
                g_f32 = g_t[:]
                if _X_NOMULT:
                    gv_tiles[ci] = g_t
                    return
                gv_t = gvpool.tile([P, (cellmax // P) * DF], f32, tag="gv")
                nc.vector.tensor_tensor(
                    out=gv_t[:, :ncol * DF].rearrange("p (c f) -> p c f", f=DF),
                    in0=g_f32[:, :ncol * DF].rearrange("p (c f) -> p c f", f=DF),
                    in1=val_t[:].to_broadcast([P, ncol, DF]),
                    op=mybir.AluOpType.mult,
                )
                gv_tiles[ci] = gv_t

            def emit_scatters(ci):
                nonlocal n_scat
                w, (s, base, cap, rlist) = all_cells[ci]
                gix_t, six_t, val_t = cell_tiles[ci]
                gv_t = gv_tiles.pop(ci)
                if _X_NOSCAT:
                    cum_scat[ci] = n_scat
                    return
                if _X_NOMULT:
                    gv_t = gv_t[:].bitcast(f32)
                for (rpos, rcap) in rlist:
                    o = rpos - base
                    prev = n_scat
                    nc.gpsimd.dma_scatter_add(
                        out_ap=outw[w][:, :],
                        in_ap=gv_t[:, (o // P) * DF:((o + rcap) // P) * DF]
                            .rearrange("p (c f) -> p c f", f=DF),
                        idxs_ap=six_t[:, o // 16:(o + rcap) // 16],
                        num_idxs=rcap,
                        num_idxs_reg=rcap,
                        elem_size=DF,
                        single_packet=False,
                    ).then_inc(scat_sem, 16)._maybe_wait_ge(
                        (scat_sem, 16 * prev) if prev > 0 else None)
                    n_scat += 1
                cum_scat[ci] = n_scat

            n_cells = len(all_cells)
            next_load = 0
            for ci in range(n_cells):
                while next_load <= min(ci + PF, n_cells - 1):
                    emit_loads(next_load)
                    next_load += 1
                emit_gather(ci)
                if ci > 0:
                    emit_scatters(ci - 1)
            if all_cells:
                emit_scatters(n_cells - 1)

    nc.compile()
    return nc


def _grid_key(plan):
    return (plan["totpos"],
            tuple((w, s, base, cap, tuple(rl))
                  for w, cells in enumerate(plan["windows"])
                  for (s, base, cap, rl) in cells))


def _get_program(num_e, plan):
    key = (num_e, _grid_key(plan))
    if key not in _prog_cache:
        _prog_cache[key] = _build_program(num_e, plan)
    return _prog_cache[key]


# ---------------------------------------------------------------------------
# Entry point
# ---------------------------------------------------------------------------

def kernel(simplex_features, boundary_values, boundary_rows, boundary_cols,
           num_out, _trace=False):
    from concourse.bass_utils import run_bass_kernel_spmd

    num_out = int(num_out)
    feats = np.ascontiguousarray(np.asarray(simplex_features, np.float32))
    num_e = feats.shape[0]

    plan, gidx_arr, sidx_arr, vals_arr = _plan(
        np.asarray(boundary_rows), np.asarray(boundary_cols),
        np.asarray(boundary_values), num_out, num_e)

    nc = _get_program(num_e, plan)

    in_maps = [
        {
            "features": feats,
            "gidx": np.ascontiguousarray(gidx_arr[i]),
            "sidx": np.ascontiguousarray(sidx_arr[i]),
            "vals": np.ascontiguousarray(vals_arr[i]),
        }
        for i in range(N_CORES)
    ]
    res = run_bass_kernel_spmd(nc, in_maps, list(range(N_CORES)), trace=_trace)

    rpc = plan["rows_per_core"]
    out = np.empty((num_out, DF), np.float32)
    for i in range(N_CORES):
        parts = []
        rem = rpc
        for w in range(plan["n_win"]):
            take = min(WIN, rem)
            parts.append(np.asarray(res.results[i][f"out_w{w}"])[:take])
            rem -= take
        # rows were rebalanced across cores; scatter back to true positions
        out[plan["row_ids_per_core"][i]] = np.concatenate(parts, axis=0)
    if _trace:
        return out, res
    return out


def estimate_core_time_ns(simplex_features, boundary_values, boundary_rows,
                          boundary_cols, num_out):
    """Cost-model span (ns) of one core's program via no-exec CoreSim."""
    from concourse.bass_interp import CoreSim

    num_out = int(num_out)
    num_e = np.asarray(simplex_features).shape[0]
    plan, _, _, _ = _plan(
        np.asarray(boundary_rows), np.asarray(boundary_cols),
        np.asarray(boundary_values), num_out, num_e)
    nc = _get_program(num_e, plan)
    sim = CoreSim(nc, no_exec=True, publish_trace=False)
    sim.simulate()
    return int(sim.time)



# revision 35
# speedup vs baseline: 1.0217x; 1.0030x over previous
"""Trainium2 Bass kernel for BoundaryOperator SpMM (gnn_message_passing), v4.

out[r, :] = sum over nnz (r, c, v): v * features[c, :].  3M nnz, 500k output
rows, 64 features, 8 cores.

Architecture (row-sharded, SWDGE gather/scatter, no matmul):
  - Core i owns output rows [i*62500, (i+1)*62500) and the ~375k nnz whose
    row lands there.  Output rows split into 2 windows of <=32768 rows so
    scatter indices fit int16.
  - Within a window, nnz are grouped into CELLS by 32768-row feature
    segment (col >> 15), so one `dma_gather` per cell fetches all feature
    rows with int16 indices against a per-cell segment base.
  - Gathered rows are multiplied by the nnz values on DVE (one batched
    tensor_tensor per cell).
  - Each cell's entries are split into occurrence-RANK groups (k-th entry
    of a row in the cell -> group k) so every `dma_scatter_add` has unique
    target rows; scatters are chained on a DMA-completion semaphore so no
    two scatters (which may share rows) are in flight together.
  - Scatter pads point at distinct unused rows with value 0 (adds 0.0).
  - All 8 cores share one instruction grid (per-cell/rank caps = max over
    cores), keeping the program SPMD; per-core index/value streams differ.
  - v4: index/value streams are loaded per CELL in small chunks prefetched
    on the SP queue (the v2 monolithic per-window loads stalled Pool ~20us
    at the start and at the window transition); deeper g/gv pools.

Cost model (measured on CoreSim): every instruction is charged
free_size * CYCLE_T[engine] on its issuing engine; SWDGE gather/scatter
cost 0.4167 ns/idx (64 f32 elements / 128 partitions * 0.833ns) on Pool,
serializing there (~351us busy for totpos=421k); the DVE multiply (~225us)
and all DMA-copy transfers hide underneath.  Notes for future work:
  - uint64-view gathers (32 elems/row = 0.208 ns/idx in the model) produce
    GARBAGE on real HW - the Q7 SWDGE kernel does not support 8B dtypes.
  - The remaining headroom is grid padding (421k vs 375k raw nnz per core):
    per-(cell, rank) caps are max-over-cores rounded to 128.  A doubled
    [F; -F] table gives each nnz two candidate segments (kernel_mirror_wip
    .py): balancing flips can equalize cells, but leftover same-(row,seg)
    collisions still cost ~128-entry rank blocks per cell; eliminating the
    rank machinery entirely requires same-channel (pos mod 16) placement of
    duplicate rows within one scatter - untested on HW.
"""

import os
import sys

import numpy as np

if "/opt/trn_rl_repo" not in sys.path:
    sys.path.append("/opt/trn_rl_repo")

# timing experiments (CoreSim only; numerically wrong when set)
_X_NOZERO = bool(int(os.environ.get("KX_NOZERO", "0")))
_X_NOMULT = bool(int(os.environ.get("KX_NOMULT", "0")))
_X_NOSCAT = bool(int(os.environ.get("KX_NOSCAT", "0")))

P = 128
DF = 64
N_CORES = 8
WIN = 32768          # rows per scatter window (int16 index range)
SEG = 32768          # feature rows per gather segment

_prog_cache: dict = {}


# ---------------------------------------------------------------------------
# Host-side planning
# ---------------------------------------------------------------------------

def _balance_rows(rows, seg, num_out, rows_per_core, n_win, n_seg):
    """Assign output rows to (core, win) bins equalizing per-(win, seg)
    cell counts across cores.

    The SPMD grid caps each (win, seg, rank) block at the max count over
    cores (rounded to 128); random row->core assignment costs ~+3 sigma of
    padding per cell.  Swapping rows between over- and under-loaded cores
    (same window, preserving bin sizes) flattens the per-seg counts, so the
    shared caps approach the mean.  Returns (core_of_row, win_of_row,
    wrow_of_row, row_ids_per_core) with row_ids in window-major order.
    """
    win_sizes = [min(WIN, rows_per_core - w * WIN) for w in range(n_win)]
    # natural init
    r_all = np.arange(num_out, dtype=np.int64)
    core_of = r_all // rows_per_core
    lrow = r_all - core_of * rows_per_core
    win_of = lrow >> 15
    bin_of = core_of * n_win + win_of
    nbins = N_CORES * n_win

    deg = np.bincount(rows, minlength=num_out)
    low_deg = deg <= 3          # swap candidates: little collateral noise
    rng = np.random.default_rng(99)
    for rnd in range(24):
        bid = bin_of[rows]
        cnt = np.bincount(bid * n_seg + seg,
                          minlength=nbins * n_seg).reshape(nbins, n_seg)
        moved = 0
        for w in range(n_win):
            bins = np.arange(N_CORES) * n_win + w
            M = cnt[bins]
            spread = M.max(axis=0) - M.min(axis=0)
            for s in np.argsort(-spread)[:24]:
                gap = int(spread[s])
                if gap < 8:
                    break
                ob = int(bins[M[:, s].argmax()])
                ub = int(bins[M[:, s].argmin()])
                k = min(gap // 2, 96)
                # low-degree rows in the over-bin touching seg s
                em = (seg == s) & (bid == ob)
                cand_o = np.unique(rows[em])
                # bid is stale within a round: keep only rows still in ob
                cand_o = cand_o[low_deg[cand_o] & (bin_of[cand_o] == ob)]
                if len(cand_o) == 0:
                    continue
                k = min(k, len(cand_o))
                sel_o = rng.choice(cand_o, size=k, replace=False)
                # low-degree rows in the under-bin not touching seg s
                rows_u = np.flatnonzero((bin_of == ub) & low_deg)
                with_s = np.unique(rows[(seg == s) & (bid == ub)])
                cand_u = rows_u[~np.isin(rows_u, with_s)]
                if len(cand_u) < k:
                    k = len(cand_u)
                    sel_o = sel_o[:k]
                if k == 0:
                    continue
                sel_u = rng.choice(cand_u, size=k, replace=False)
                bin_of[sel_o] = ub
                bin_of[sel_u] = ob
                moved += k
                # counts refresh at the top of the next round; stale M
                # within a round only mildly misdirects later segs
        if moved == 0:
            break

    core_of = bin_of // n_win
    win_of = bin_of - core_of * n_win
    wrow_of = np.zeros(num_out, np.int64)
    row_ids_per_core = []
    for c in range(N_CORES):
        ids_c = []
        for w in range(n_win):
            ids = np.flatnonzero(bin_of == c * n_win + w)
            assert len(ids) == win_sizes[w], (c, w, len(ids), win_sizes[w])
            wrow_of[ids] = np.arange(len(ids))
            ids_c.append(ids)
        row_ids_per_core.append(np.concatenate(ids_c))
    return core_of, win_of, wrow_of, row_ids_per_core


def _plan(rows, cols, vals, num_out, num_e):
    rows = np.asarray(rows, np.int64)
    cols = np.asarray(cols, np.int64)
    vals = np.asarray(vals, np.float32)
    rows_per_core = num_out // N_CORES
    assert num_out % N_CORES == 0
    n_win = -(-rows_per_core // WIN)
    n_seg = -(-num_e // SEG)

    seg = cols >> 15
    segcol = cols & (SEG - 1)
    core_of, win_of, wrow_of, row_ids_per_core = _balance_rows(
        rows, seg, num_out, rows_per_core, n_win, n_seg)
    core = core_of[rows]
    win = win_of[rows]
    wrow = wrow_of[rows]

    # Per (core, win, seg) cell: entries with per-row occurrence rank.
    # Cells are split into sub-cells of <= CAPR raw entries to bound the
    # SWDGE descriptor-ring occupancy per instruction.
    CAPR = 10 ** 9   # no sub-cell split (single_packet=False handles big cells)
    order = np.lexsort((rows, seg, win, core))
    c_s, w_s, s_s, r_s = core[order], win[order], seg[order], rows[order]
    sc_s, wr_s, v_s = segcol[order], wrow[order], vals[order]
    idx_all = np.arange(len(order))
    # sub-cell = index within the (core, win, seg) group // CAPR
    cell_change = np.ones(len(order), bool)
    cell_change[1:] = (s_s[1:] != s_s[:-1]) | (w_s[1:] != w_s[:-1]) | \
                      (c_s[1:] != c_s[:-1])
    cell_id = np.cumsum(cell_change) - 1
    cell_starts = np.flatnonzero(cell_change)
    sub_s = (idx_all - cell_starts[cell_id]) // CAPR
    n_sub = int(sub_s.max()) + 1
    # fold sub into the segment key for grid purposes
    k_s = s_s * n_sub + sub_s
    n_key = n_seg * n_sub
    # run starts where (core, win, key, row) changes
    key_change = np.ones(len(order), bool)
    key_change[1:] = (r_s[1:] != r_s[:-1]) | (k_s[1:] != k_s[:-1]) | \
                     (w_s[1:] != w_s[:-1]) | (c_s[1:] != c_s[:-1])
    run_id = np.cumsum(key_change) - 1
    run_starts = np.flatnonzero(key_change)
    rank_s = idx_all - run_starts[run_id]

    # counts[core, win, key, rank]
    max_rank = int(rank_s.max()) + 1
    cnt = np.zeros((N_CORES, n_win, n_key, max_rank), np.int64)
    np.add.at(cnt, (c_s, w_s, k_s, rank_s), 1)

    # Common grid: per (win, key, rank) cap = max over cores, 128-aligned.
    caps = cnt.max(axis=0)                       # [n_win, n_key, max_rank]
    caps = ((caps + P - 1) // P) * P

    # Instruction tables (identical across cores).
    # Per window: list of cells (seg, pos_base, cell_cap, [(rank_cap, pos)]).
    # Cells are emitted largest-first so the pipeline tail (the last cell's
    # scatter chain after the final gather) is as short as possible.
    kord = [np.argsort(-caps[w].sum(axis=1), kind="stable")
            for w in range(n_win)]
    windows = []
    pos = 0
    for w in range(n_win):
        cells = []
        for k in kord[w]:
            rlist = []
            base = pos
            for r in range(max_rank):
                cap = int(caps[w, k, r])
                if cap == 0:
                    break
                rlist.append((pos, cap))
                pos += cap
            if rlist:
                cells.append((int(k) // n_sub, base, pos - base, rlist))
        windows.append(cells)
    totpos = pos
    assert totpos % P == 0

    # Per-core streams.
    gidx = np.zeros((N_CORES, totpos), np.int16)
    sidx = np.zeros((N_CORES, totpos), np.int16)
    valp = np.zeros((N_CORES, totpos), np.float32)

    # Position of each sorted nnz: pos_base(core,w,s,rank) + index-within.
    # Build per-(c,w,s,r) base table from grid caps.
    base_tab = np.zeros((n_win, n_key, max_rank), np.int64)
    pos2 = 0
    for w in range(n_win):
        for k in kord[w]:
            for r in range(max_rank):
                cap = int(caps[w, k, r])
                if cap == 0:
                    break
                base_tab[w, k, r] = pos2
                pos2 += cap
    # index within (c,w,s,r) group: entries are consecutive in sorted order
    # for fixed (c,w,s) with rank increasing within each row-run; regroup via
    # a second sort by (core, win, seg, rank).
    order2 = np.lexsort((rank_s, k_s, w_s, c_s))
    c2, w2, s2, r2 = c_s[order2], w_s[order2], k_s[order2], rank_s[order2]
    sc2, wr2, v2 = sc_s[order2], wr_s[order2], v_s[order2]
    grp_change = np.ones(len(order2), bool)
    grp_change[1:] = (r2[1:] != r2[:-1]) | (s2[1:] != s2[:-1]) | \
                     (w2[1:] != w2[:-1]) | (c2[1:] != c2[:-1])
    g_id = np.cumsum(grp_change) - 1
    g_starts = np.flatnonzero(grp_change)
    within = idx_all - g_starts[g_id]
    p2 = base_tab[w2, s2, r2] + within
    gidx[c2, p2] = sc2
    sidx[c2, p2] = wr2
    valp[c2, p2] = v2

    # Scatter pads: for each (core, win, seg, rank) group, pad slots get
    # distinct rows unused by that group (value stays 0 -> adds 0.0).
    filled = np.zeros((N_CORES, n_win, n_key, max_rank), np.int64)
    np.add.at(filled, (c2, w2, s2, r2), 1)
    for c in range(N_CORES):
        for w in range(n_win):
            for s in range(n_key):
                for r in range(max_rank):
                    cap = int(caps[w, s, r])
                    if cap == 0:
                        break
                    nfill = int(filled[c, w, s, r])
                    npad = cap - nfill
                    if npad == 0:
                        continue
                    b = base_tab[w, s, r]
                    used = sidx[c, b:b + nfill]
                    # candidate pad rows from the top of the window
                    cand = np.arange(WIN - 1, WIN - 1 - nfill - npad - 8, -1,
                                     dtype=np.int64)
                    cand = cand[~np.isin(cand, used)][:npad]
                    sidx[c, b + nfill:b + cap] = cand.astype(np.int16)

    # Wrap streams into device layouts.
    # idx: [128, totpos/16] int16, replicated per 16-partition group.
    g16 = gidx.reshape(N_CORES, totpos // 16, 16).transpose(0, 2, 1)
    s16 = sidx.reshape(N_CORES, totpos // 16, 16).transpose(0, 2, 1)
    gidx_arr = np.tile(g16, (1, 8, 1)).copy()       # [N, 128, totpos/16]
    sidx_arr = np.tile(s16, (1, 8, 1)).copy()
    vals_arr = valp.reshape(N_CORES, totpos // P, P).transpose(0, 2, 1).copy()

    plan = {
        "windows": windows,
        "totpos": totpos,
        "n_win": n_win,
        "rows_per_core": rows_per_core,
        "row_ids_per_core": row_ids_per_core,
        "cellcap_max": max((cc for cells in windows for _, _, cc, _ in cells),
                           default=P),
        "win_pos": [  # per window: (pos_base, npos)
        ],
    }
    for w, cells in enumerate(windows):
        if cells:
            b0 = cells[0][1]
            b1 = cells[-1][1] + cells[-1][2]
            plan["win_pos"].append((b0, b1 - b0))
        else:
            plan["win_pos"].append((0, 0))
    return plan, gidx_arr, sidx_arr, vals_arr


# ---------------------------------------------------------------------------
# Bass program
# ---------------------------------------------------------------------------

def _build_program(num_e, plan):
    import concourse.bacc as bacc
    import concourse.bass as bass
    import concourse.mybir as mybir
    from concourse.tile import TileContext

    f32, i16 = mybir.dt.float32, mybir.dt.int16
    totpos = plan["totpos"]
    windows = plan["windows"]
    cellmax = plan["cellcap_max"]
    out_rows = 2 * WIN      # window 1 scatter range may reach 65535
    zero_rows = ((plan["rows_per_core"] + P - 1) // P) * P

    nc = bacc.Bacc(None, target_bir_lowering=False,
                   dynamic_dma_scratch_size=49152)
    feat = nc.dram_tensor("features", [num_e, DF], f32, kind="ExternalInput")
    gidxd = nc.dram_tensor("gidx", [P, totpos // 16], i16, kind="ExternalInput")
    sidxd = nc.dram_tensor("sidx", [P, totpos // 16], i16, kind="ExternalInput")
    valsd = nc.dram_tensor("vals", [P, totpos // P], f32, kind="ExternalInput")
    outw = [nc.dram_tensor(f"out_w{w}", [WIN, DF], f32, kind="ExternalOutput")
            for w in range(len(windows))]

    GB = 8     # gather-tile pool depth
    GVB = 6    # multiplied-tile pool depth (slots read by scatter DMAs)
    SB = 12    # per-cell stream-chunk pool depth
    with TileContext(nc) as tc:
        with (
            tc.tile_pool(name="z", bufs=1) as zpool,
            tc.tile_pool(name="idx", bufs=SB) as ipool,
            tc.tile_pool(name="g", bufs=GB) as gpool,
            tc.tile_pool(name="gv", bufs=GVB) as gvpool,
        ):
            # Zero the real output rows (Act engine, big contiguous DMAs).
            # Per-window tensors: window 1 scatters only wait on window-1
            # zeroing, which overlaps with window-0 processing.
            z_t = zpool.tile([P, 2048], f32)
            nc.vector.memset(z_t[:], 0.0)
            zrem = zero_rows
            for w in range(len(windows)):
                zr = min(WIN, ((zrem + P - 1) // P) * P)
                if zr <= 0:
                    break
                zrem -= zr
                zc = (zr // P) * DF
                ov = outw[w].ap()[0:zr, :].rearrange("(p c) f -> p (c f)", p=P)
                for o in range(0, zc, 2048):
                    ww = min(2048, zc - o)
                    if not _X_NOZERO:
                        nc.scalar.dma_start(out=ov[:, o:o + ww], in_=z_t[:, :ww])

            scat_sem = nc.alloc_semaphore("scat_dma")
            n_scat = 0

            # Software pipeline: gather(cell i+1) is emitted before the
            # scatters of cell i so the Pool engine overlaps gathers with
            # scatter sem-waits.  Index/value streams are loaded per CELL
            # (small chunks, prefetched PF cells ahead on the SP queue) so
            # the Pool engine never stalls on a monolithic window load.
            all_cells = [(w, cell) for w, cells in enumerate(windows)
                         for cell in cells]
            cell_tiles = {}
            gv_tiles = {}
            cum_scat = {}   # cell index -> total scatters emitted through it
            PF = SB - 2

            def emit_loads(ci):
                w, (s, base, cap, rlist) = all_cells[ci]
                # stream-chunk slot reused from cell ci-SB: six/val are read
                # by that cell's scatter DMAs -> gate the overwrite on their
                # completion (SP engine wait; SP is otherwise idle).
                if ci >= SB and cum_scat[ci - SB] > 0:
                    nc.sync.wait_ge(scat_sem, 16 * cum_scat[ci - SB])
                gix_t = ipool.tile([P, cap // 16], i16, tag="gix")
                nc.sync.dma_start(out=gix_t[:],
                                  in_=gidxd[:, base // 16:(base + cap) // 16])
                six_t = ipool.tile([P, cap // 16], i16, tag="six")
                nc.sync.dma_start(out=six_t[:],
                                  in_=sidxd[:, base // 16:(base + cap) // 16])
                val_t = ipool.tile([P, cap // P], f32, tag="val")
                nc.sync.dma_start(out=val_t[:],
                                  in_=valsd[:, base // P:(base + cap) // P])
                cell_tiles[ci] = (gix_t, six_t, val_t)

            def emit_gather(ci):
                w, (s, base, cap, rlist) = all_cells[ci]
                gix_t, six_t, val_t = cell_tiles[ci]
                # gv slot reused by cell ci from cell ci-GVB, whose scatter
                # DMAs read it asynchronously: gate the DVE multiply on their
                # completion.
                if ci >= GVB:
                    nc.vector.wait_ge(scat_sem, 16 * cum_scat[ci - GVB])
                ncol = cap // P
                g_t = gpool.tile([P, (cellmax // P) * DF], f32, tag="g")
                seg_lo = s * SEG
                seg_hi = min(seg_lo + SEG, num_e)
                nc.gpsimd.dma_gather(
                    out_ap=g_t[:, :ncol * DF].rearrange("p (c f) -> p c f",
                                                        f=DF),
                    in_ap=feat[seg_lo:seg_hi, :],
                    idxs_ap=gix_t[:],
                    num_idxs=cap,
                    num_idxs_reg=cap,
                    elem_size=DF,
                    single_packet=False,
                )
                g_f32 = g_t[:]
                if _X_NOMULT:
                    gv_tiles[ci] = g_t
                    return
                gv_t = gvpool.tile([P, (cellmax // P) * DF], f32, tag="gv")
                nc.vector.tensor_tensor(
                    out=gv_t[:, :ncol * DF].rearrange("p (c f) -> p c f", f=DF),
                    in0=g_f32[:, :ncol * DF].rearrange("p (c f) -> p c f", f=DF),
                    in1=val_t[:].to_broadcast([P, ncol, DF]),
                    op=mybir.AluOpType.mult,
                )
                gv_tiles[ci] = gv_t

            def emit_scatters(ci):
                nonlocal n_scat
                w, (s, base, cap, rlist) = all_cells[ci]
                gix_t, six_t, val_t = cell_tiles[ci]
                gv_t = gv_tiles.pop(ci)
                if _X_NOSCAT:
                    cum_scat[ci] = n_scat
                    return
                if _X_NOMULT:
                    gv_t = gv_t[:].bitcast(f32)
                for (rpos, rcap) in rlist:
                    o = rpos - base
                    prev = n_scat
                    nc.gpsimd.dma_scatter_add(
                        out_ap=outw[w][:, :],
                        in_ap=gv_t[:, (o // P) * DF:((o + rcap) // P) * DF]
                            .rearrange("p (c f) -> p c f", f=DF),
                        idxs_ap=six_t[:, o // 16:(o + rcap) // 16],
                        num_idxs=rcap,
                        num_idxs_reg=rcap,
                        elem_size=DF,
                        single_packet=False,
                    ).then_inc(scat_sem, 16)._maybe_wait_ge(
                        (scat_sem, 16 * prev) if prev > 0 else None)
                    n_scat += 1
                cum_scat[ci] = n_scat

            n_cells = len(all_cells)
            next_load = 0
            for ci in range(n_cells):
                while next_load <= min(ci + PF, n_cells - 1):
                    emit_loads(next_load)
                    next_load += 1
                emit_gather(ci)
                if ci > 0:
                    emit_scatters(ci - 1)
            if all_cells:
                emit_scatters(n_cells - 1)

    nc.compile()
    return nc


def _grid_key(plan):
    return (plan["totpos"],
            tuple((w, s, base, cap, tuple(rl))
                  for w, cells in enumerate(plan["windows"])
                  for (s, base, cap, rl) in cells))


def _get_program(num_e, plan):
    key = (num_e, _grid_key(plan))
    if key not in _prog_cache:
        _prog_cache[key] = _build_program(num_e, plan)
    return _prog_cache[key]


# ---------------------------------------------------------------------------
# Entry point
# ---------------------------------------------------------------------------

def kernel(simplex_features, boundary_values, boundary_rows, boundary_cols,
           num_out, _trace=False):
    from concourse.bass_utils import run_bass_kernel_spmd

    num_out = int(num_out)
    feats = np.ascontiguousarray(np.asarray(simplex_features, np.float32))
    num_e = feats.shape[0]

    plan, gidx_arr, sidx_arr, vals_arr = _plan(
        np.asarray(boundary_rows), np.asarray(boundary_cols),
        np.asarray(boundary_values), num_out, num_e)

    nc = _get_program(num_e, plan)

    in_maps = [
        {
            "features": feats,
            "gidx": np.ascontiguousarray(gidx_arr[i]),
            "sidx": np.ascontiguousarray(sidx_arr[i]),
            "vals": np.ascontiguousarray(vals_arr[i]),
        }
        for i in range(N_CORES)
    ]
    res = run_bass_kernel_spmd(nc, in_maps, list(range(N_CORES)), trace=_trace)

    rpc = plan["rows_per_core"]
    out = np.empty((num_out, DF), np.float32)
    for i in range(N_CORES):
        parts = []
        rem = rpc
        for w in range(plan["n_win"]):
            take = min(WIN, rem)
            parts.append(np.asarray(res.results[i][f"out_w{w}"])[:take])
            rem -= take
        # rows were rebalanced across cores; scatter back to true positions
        out[plan["row_ids_per_core"][i]] = np.concatenate(parts, axis=0)
    if _trace:
        return out, res
    return out


def estimate_core_time_ns(simplex_features, boundary_values, boundary_rows,
                          boundary_cols, num_out):
    """Cost-model span (ns) of one core's program via no-exec CoreSim."""
    from concourse.bass_interp import CoreSim

    num_out = int(num_out)
    num_e = np.asarray(simplex_features).shape[0]
    plan, _, _, _ = _plan(
        np.asarray(boundary_rows), np.asarray(boundary_cols),
        np.asarray(boundary_values), num_out, num_e)
    nc = _get_program(num_e, plan)
    sim = CoreSim(nc, no_exec=True, publish_trace=False)
    sim.simulate()
    return int(sim.time)

